# revision 1
# baseline (speedup 1.0000x reference)
"""Trainium2 Bass kernel for the NodeEdge GNN message-passing module.

Computes  out[b,n,h] = sum_e (w*inci + b)[n,e] * relu(inputs @ W_xes + b_xes)[b,e,h]
with B=16, N=2048, E=8192, DIM=64, DH=32.

Strategy: shard the edge (contraction) dimension E across the 8 NeuronCores
(EC=1024 edges per core). Each core:
  - computes xe = relu(inputs[:, e_shard, :] @ W_xes) for its edge shard
    in [e, (b,h)] layout via small PE matmuls,
  - forms A^T chunks (w * inci, transposed so e is the partition axis;
    the transpose itself is done on the host as layout prep),
  - runs the big matmul  out_partial[(b,h), n] = xe^T @ A^T  with fp32r
    (FP22 reduced-precision, full PE rate) accumulating in f32 PSUM.
Partial outputs (one per core) are summed on the host.

inci is shipped as uint8 (2 MiB of HBM traffic per core instead of 8),
cast to f32 on the vector engine, and multiplied into the w chunks in
place. Matmul operands are declared float32r end to end (the BIR
verifier requires fp32r matmul inputs to be produced as fp32r).
"""

from contextlib import ExitStack

import numpy as np

import concourse.bass as bass
import concourse.mybir as mybir
import concourse.tile as tile
from concourse import bacc
from concourse.bass_utils import run_bass_kernel_spmd

B, N, E, DIM = 16, 2048, 8192, 64
DH = DIM // 2              # 32
NCORES = 8
EC = E // NCORES           # 1024 edges per core
KC = EC // 128             # 8 e-chunks of 128
BH = B * DH                # 512 (flattened (b, h) output dim)
NB = N // 512              # 4 column blocks of the big matmul
NJ = B // 2                # 8 input tiles, two batch rows packed per tile

F32 = mybir.dt.float32
F32R = mybir.dt.float32r
U8 = mybir.dt.uint8

_PROGRAMS: dict = {}


def _build_program(with_bxes: bool, with_b: bool):
    nc = bacc.Bacc(
        "TRN2", target_bir_lowering=False, debug=False, enable_asserts=False
    )

    inp_t = nc.dram_tensor("inp_t", [NJ, 128, EC], F32R, kind="ExternalInput").ap()
    wq = nc.dram_tensor("wq", [KC, 128, N], F32R, kind="ExternalInput").ap()
    iq = nc.dram_tensor("iq", [KC, 128, N], U8, kind="ExternalInput").ap()
    wx = nc.dram_tensor("wx", [128, 2 * DH], F32R, kind="ExternalInput").ap()
    bxr = (
        nc.dram_tensor("bxr", [128, BH], F32, kind="ExternalInput").ap()
        if with_bxes
        else None
    )
    bq = (
        nc.dram_tensor("bq", [KC, 128, N], F32, kind="ExternalInput").ap()
        if with_b
        else None
    )
    outp = nc.dram_tensor("outp", [BH, N], F32, kind="ExternalOutput").ap()

    with tile.TileContext(nc) as tc, ExitStack() as ctx:
        inp_pool = ctx.enter_context(tc.tile_pool(name="inp", bufs=NJ))
        wx_pool = ctx.enter_context(tc.tile_pool(name="wx", bufs=1))
        xe_pool = ctx.enter_context(tc.tile_pool(name="xe", bufs=KC))
        a_pool = ctx.enter_context(tc.tile_pool(name="a", bufs=KC))
        i_pool = ctx.enter_context(tc.tile_pool(name="i", bufs=3))
        out_pool = ctx.enter_context(tc.tile_pool(name="o", bufs=8))
        ps_pool = ctx.enter_context(tc.tile_pool(name="ps", bufs=8, space="PSUM"))

        # Block-diagonal xes weight: rows 0-63 map the even batch row to
        # output cols 0-31, rows 64-127 map the odd batch row to cols
        # 32-63, so one K=128 matmul computes xe for both packed batch
        # rows of an input tile at once.
        wx_tile = wx_pool.tile([128, 2 * DH], F32R)
        nc.sync.dma_start(wx_tile[:], wx[:])

        # incidence loads first: they are small and the A^T chain needs
        # them early (cast runs while PE is busy with xe)
        iu_tiles = []
        for k in range(KC):
            iu = i_pool.tile([128, N], U8, tag="iu", name=f"iu_{k}", bufs=KC)
            nc.gpsimd.dma_start(iu[:], iq[k])
            iu_tiles.append(iu)

        bx_tile = None
        if with_bxes:
            bx_tile = wx_pool.tile([128, BH], F32, tag="bx")
            nc.sync.dma_start(bx_tile[:], bxr[:])

        # ---- load inputs (two batch rows packed per 128-partition tile) ----
        inp_tiles = []
        for j in range(NJ):
            t = inp_pool.tile([128, EC], F32R)
            nc.sync.dma_start(t[:], inp_t[j])
            inp_tiles.append(t)

        # ---- xe = relu(inputs @ W_xes) in [e, (b,h)] layout ----
        xe_tiles = []
        for k in range(KC):
            ps = ps_pool.tile([128, BH], F32, tag="ps")
            for j in range(NJ):
                lhsT = inp_tiles[j][:, k * 128 : (k + 1) * 128]
                nc.tensor.matmul(
                    ps[:, j * 2 * DH : (j + 1) * 2 * DH],
                    lhsT,
                    wx_tile[:],
                    start=True,
                    stop=True,
                )
            xt = xe_pool.tile([128, BH], F32R)
            if with_bxes:
                nc.vector.tensor_tensor(
                    xt[:], ps[:], bx_tile[:], op=mybir.AluOpType.add
                )
                nc.scalar.activation(
                    xt[:], xt[:], mybir.ActivationFunctionType.Relu
                )
            else:
                nc.scalar.activation(
                    xt[:], ps[:], mybir.ActivationFunctionType.Relu
                )
            xe_tiles.append(xt)

        # ---- A^T chunks: w, then *= inci. The u8 -> f32 casts run on
        # GpSimd (otherwise idle) so the DVE FIFO carries only the
        # w-DMA-paced multiply chain, and they are emitted chunks ahead.
        # Each chunk is split into NB quarter-tiles (one per output
        # column block) so multiplies and dependent matmuls start on the
        # first quarter while the rest of the w DMA is still in flight.
        QN = N // NB  # 512, one tile per output column block
        a_tiles = []
        it_tiles = [None] * KC
        for k in range(KC):
            ah = [
                a_pool.tile([128, QN], F32R, tag="a", bufs=NB * KC,
                            name=f"a_{k}_{h}")
                for h in range(NB)
            ]
            for h in range(NB):
                nc.sync.dma_start(ah[h][:], wq[k][:, h * QN : (h + 1) * QN])
            a_tiles.append(ah)

        def emit_cast(k):
            it = i_pool.tile([128, N], F32, tag="it", name=f"it_{k}", bufs=3)
            nc.gpsimd.tensor_copy(it[:], iu_tiles[k][:])
            it_tiles[k] = it

        emit_cast(0)
        if KC > 1:
            emit_cast(1)
        if KC > 2:
            emit_cast(2)
        for k in range(KC):
            for h in range(NB):
                nc.vector.tensor_tensor(
                    a_tiles[k][h][:], a_tiles[k][h][:],
                    it_tiles[k][:, h * QN : (h + 1) * QN],
                    op=mybir.AluOpType.mult,
                )
            if k + 3 < KC:
                emit_cast(k + 3)
            if with_b:
                bt = i_pool.tile([128, N], F32, tag="bt", bufs=2)
                nc.sync.dma_start(bt[:], bq[k])
                for h in range(NB):
                    nc.vector.tensor_tensor(
                        a_tiles[k][h][:], a_tiles[k][h][:],
                        bt[:, h * QN : (h + 1) * QN],
                        op=mybir.AluOpType.add,
                    )

        # ---- big matmul: out[(b,h), n] += xe^T @ A^T, fp32r, f32 accum ----
        for pair in range(BH // 256):  # two (b,h) 128-chunks at a time
            pstiles = [
                [
                    ps_pool.tile(
                        [128, 512], F32, tag="ps", name=f"bps_{pair}_{h2}_{nb}"
                    )
                    for nb in range(NB)
                ]
                for h2 in range(2)
            ]
            if pair == 0:
                # arrival-paced: walk k outermost so each chunk is used
                # as soon as its A^T half lands
                for k in range(KC):
                    for half in range(2):
                        bh = 2 * pair + half
                        lhsT = xe_tiles[k][:, bh * 128 : (bh + 1) * 128]
                        for nb in range(NB):
                            nc.tensor.matmul(
                                pstiles[half][nb][:],
                                lhsT,
                                a_tiles[k][nb][:],
                                start=(k == 0),
                                stop=(k == KC - 1),
                            )
            else:
                # all data has arrived by now: walk half outermost so the
                # first 4 groups close early and their copies/stores
                # overlap the remaining matmuls
                for half in range(2):
                    bh = 2 * pair + half
                    for nb in range(NB):
                        for k in range(KC):
                            nc.tensor.matmul(
                                pstiles[half][nb][:],
                                xe_tiles[k][:, bh * 128 : (bh + 1) * 128],
                                a_tiles[k][nb][:],
                                start=(k == 0),
                                stop=(k == KC - 1),
                            )
                    for nb in range(NB):
                        ot = out_pool.tile(
                            [128, 512], F32, tag="o", name=f"ot1_{half}_{nb}"
                        )
                        if nb % 2 == 0:
                            nc.scalar.activation(
                                ot[:],
                                pstiles[half][nb][:],
                                mybir.ActivationFunctionType.Identity,
                            )
                        else:
                            nc.vector.tensor_copy(ot[:], pstiles[half][nb][:])
                        nc.scalar.dma_start(
                            outp[bh * 128 : (bh + 1) * 128,
                                 nb * 512 : (nb + 1) * 512],
                            ot[:],
                        )
                continue
            for half in range(2):
                bh = 2 * pair + half
                for nb in range(NB):
                    ot = out_pool.tile([128, 512], F32, tag="o")
                    if nb % 2 == 0:
                        nc.scalar.activation(
                            ot[:],
                            pstiles[half][nb][:],
                            mybir.ActivationFunctionType.Identity,
                        )
                    else:
                        nc.vector.tensor_copy(ot[:], pstiles[half][nb][:])
                    nc.sync.dma_start(
                        outp[bh * 128 : (bh + 1) * 128, nb * 512 : (nb + 1) * 512],
                        ot[:],
                    )

    nc.compile()
    return nc


def _get_program(with_bxes: bool, with_b: bool):
    key = (with_bxes, with_b)
    if key not in _PROGRAMS:
        _PROGRAMS[key] = _build_program(with_bxes, with_b)
    return _PROGRAMS[key]


def _prepare_in_maps(inputs, W_xes, b_xes, inci, w, b, with_bxes, with_b):
    inputs = np.ascontiguousarray(np.asarray(inputs, dtype=np.float32))
    W_xes = np.ascontiguousarray(np.asarray(W_xes, dtype=np.float32))
    b_xes = np.asarray(b_xes, dtype=np.float32)
    w = np.asarray(w, dtype=np.float32)
    b = np.asarray(b, dtype=np.float32)
    inci_u8 = np.asarray(inci).astype(np.uint8)

    wx_dup = np.zeros((128, 2 * DH), dtype=np.float32)
    wx_dup[0:DIM, 0:DH] = W_xes
    wx_dup[DIM : 2 * DIM, DH : 2 * DH] = W_xes
    bxr = np.ascontiguousarray(
        np.broadcast_to(np.tile(b_xes, B)[None, :], (128, BH))
    ) if with_bxes else None

    in_maps = []
    for c in range(NCORES):
        sl = slice(c * EC, (c + 1) * EC)
        t = np.ascontiguousarray(inputs[:, sl, :].transpose(0, 2, 1)).reshape(
            NJ, 128, EC
        )
        wq_ = np.ascontiguousarray(w[:, sl].T).reshape(KC, 128, N)
        iq_ = np.ascontiguousarray(inci_u8[:, sl].T).reshape(KC, 128, N)
        m = {"inp_t": t, "wq": wq_, "iq": iq_, "wx": wx_dup}
        if with_bxes:
            m["bxr"] = bxr
        if with_b:
            m["bq"] = np.ascontiguousarray(b[:, sl].T).reshape(KC, 128, N)
        in_maps.append(m)
    return in_maps


def _run(inputs, W_xes, b_xes, inci, w, b, **run_kwargs):
    with_bxes = bool(np.any(np.asarray(b_xes)))
    with_b = bool(np.any(np.asarray(b)))
    nc = _get_program(with_bxes, with_b)
    in_maps = _prepare_in_maps(inputs, W_xes, b_xes, inci, w, b, with_bxes, with_b)
    res = run_bass_kernel_spmd(
        nc, in_maps, core_ids=list(range(NCORES)), **run_kwargs
    )
    parts = np.stack([r["outp"] for r in res.results])  # [8, BH, N]
    out = parts.sum(axis=0)  # [BH, N]
    out = out.reshape(B, DH, N).transpose(0, 2, 1)  # [B, N, DH]
    return np.ascontiguousarray(out.astype(np.float32)), res


def kernel(inputs, W_xes, b_xes, inci, w, b):
    out, _ = _run(inputs, W_xes, b_xes, inci, w, b)
    return out



# revision 2
# speedup vs baseline: 1.5687x; 1.5687x over previous
"""Trainium2 Bass kernel for the NodeEdge GNN message-passing module.

Computes  out[b,n,h] = sum_e (w*inci + b)[n,e] * relu(inputs @ W_xes + b_xes)[b,e,h]
with B=16, N=2048, E=8192, DIM=64, DH=32.

Strategy: shard the edge (contraction) dimension E across the 8 NeuronCores
(EC=1024 edges per core). Each core:
  - computes xe = relu(inputs[:, e_shard, :] @ W_xes) for its edge shard
    in [e, (b,h)] layout via small PE matmuls,
  - forms A^T chunks (w * inci, transposed so e is the partition axis;
    the transpose itself is done on the host as layout prep),
  - runs the big matmul  out_partial[(b,h), n] = xe^T @ A^T  accumulating
    in f32 PSUM.
Partial outputs (one per core) are summed on the host.

All matmul operands travel as bf16 (PE runs bf16 at full rate vs half
rate for fp32r, and HBM traffic halves). inci is shipped as uint8 and
multiplied into the w chunks in place with a single mixed-dtype DVE
tensor_tensor (no separate cast pass, no GpSimd).
"""

from contextlib import ExitStack

import ml_dtypes
import numpy as np

import concourse.bass as bass
import concourse.mybir as mybir
import concourse.tile as tile
from concourse import bacc
from concourse.bass_utils import run_bass_kernel_spmd

B, N, E, DIM = 16, 2048, 8192, 64
DH = DIM // 2              # 32
NCORES = 8
EC = E // NCORES           # 1024 edges per core
KC = EC // 128             # 8 e-chunks of 128
BH = B * DH                # 512 (flattened (b, h) output dim)
NB = N // 512              # 4 column blocks of the big matmul
NJ = B // 2                # 8 input tiles, two batch rows packed per tile
HN = N // 2                # 1024, half-chunk width for w arrival pacing

F32 = mybir.dt.float32
BF16 = mybir.dt.bfloat16
U8 = mybir.dt.uint8

_PROGRAMS: dict = {}


def _build_program(with_bxes: bool, with_b: bool):
    nc = bacc.Bacc(
        "TRN2", target_bir_lowering=False, debug=False, enable_asserts=False
    )

    inp_t = nc.dram_tensor("inp_t", [NJ, 128, EC], BF16, kind="ExternalInput").ap()
    wq = nc.dram_tensor("wq", [KC, 128, N], BF16, kind="ExternalInput").ap()
    iq = nc.dram_tensor("iq", [KC, 128, N], U8, kind="ExternalInput").ap()
    wx = nc.dram_tensor("wx", [128, 2 * DH], BF16, kind="ExternalInput").ap()
    bxr = (
        nc.dram_tensor("bxr", [128, BH], F32, kind="ExternalInput").ap()
        if with_bxes
        else None
    )
    bq = (
        nc.dram_tensor("bq", [KC, 128, N], BF16, kind="ExternalInput").ap()
        if with_b
        else None
    )
    outp = nc.dram_tensor("outp", [BH, N], F32, kind="ExternalOutput").ap()

    with tile.TileContext(nc) as tc, ExitStack() as ctx:
        inp_pool = ctx.enter_context(tc.tile_pool(name="inp", bufs=NJ))
        wx_pool = ctx.enter_context(tc.tile_pool(name="wx", bufs=1))
        xe_pool = ctx.enter_context(tc.tile_pool(name="xe", bufs=KC))
        a_pool = ctx.enter_context(tc.tile_pool(name="a", bufs=2 * KC))
        i_pool = ctx.enter_context(tc.tile_pool(name="i", bufs=KC))
        out_pool = ctx.enter_context(tc.tile_pool(name="o", bufs=8))
        ps_pool = ctx.enter_context(tc.tile_pool(name="ps", bufs=8, space="PSUM"))

        # Block-diagonal xes weight: rows 0-63 map the even batch row to
        # output cols 0-31, rows 64-127 map the odd batch row to cols
        # 32-63, so one K=128 matmul computes xe for both packed batch
        # rows of an input tile at once.
        wx_tile = wx_pool.tile([128, 2 * DH], BF16)
        nc.sync.dma_start(wx_tile[:], wx[:])

        # incidence loads first: small, and the A^T chain needs them early
        iu_tiles = []
        for k in range(KC):
            iu = i_pool.tile([128, N], U8, tag="iu", name=f"iu_{k}", bufs=KC)
            nc.gpsimd.dma_start(iu[:], iq[k])
            iu_tiles.append(iu)

        bx_tile = None
        if with_bxes:
            bx_tile = wx_pool.tile([128, BH], F32, tag="bx")
            nc.sync.dma_start(bx_tile[:], bxr[:])

        # ---- load inputs (two batch rows packed per 128-partition tile) ----
        inp_tiles = []
        for j in range(NJ):
            t = inp_pool.tile([128, EC], BF16)
            nc.scalar.dma_start(t[:], inp_t[j])
            inp_tiles.append(t)

        # ---- w chunks stream in as halves so the mask-multiply and the
        # first big matmuls start while later halves are still in flight.
        a_tiles = []
        for k in range(KC):
            ah = [
                a_pool.tile([128, HN], BF16, tag="a", bufs=2 * KC,
                            name=f"a_{k}_{h}")
                for h in range(2)
            ]
            for h in range(2):
                nc.sync.dma_start(ah[h][:], wq[k][:, h * HN : (h + 1) * HN])
            a_tiles.append(ah)

        # ---- xe = relu(inputs @ W_xes) in [e, (b,h)] layout ----
        xe_tiles = []
        for k in range(KC):
            ps = ps_pool.tile([128, BH], F32, tag="ps")
            for j in range(NJ):
                lhsT = inp_tiles[j][:, k * 128 : (k + 1) * 128]
                nc.tensor.matmul(
                    ps[:, j * 2 * DH : (j + 1) * 2 * DH],
                    lhsT,
                    wx_tile[:],
                    start=True,
                    stop=True,
                )
            xt = xe_pool.tile([128, BH], BF16)
            if with_bxes:
                nc.vector.tensor_tensor(
                    xt[:], ps[:], bx_tile[:], op=mybir.AluOpType.add
                )
                nc.scalar.activation(
                    xt[:], xt[:], mybir.ActivationFunctionType.Relu
                )
            else:
                nc.scalar.activation(
                    xt[:], ps[:], mybir.ActivationFunctionType.Relu
                )
            xe_tiles.append(xt)

        # ---- A^T chunks: a = w * inci (single mixed-dtype pass) ----
        for k in range(KC):
            for h in range(2):
                nc.vector.tensor_tensor(
                    a_tiles[k][h][:], a_tiles[k][h][:],
                    iu_tiles[k][:, h * HN : (h + 1) * HN],
                    op=mybir.AluOpType.mult,
                )
            if with_b:
                bt = i_pool.tile([128, N], BF16, tag="bt", bufs=2)
                nc.sync.dma_start(bt[:], bq[k])
                for h in range(2):
                    nc.vector.tensor_tensor(
                        a_tiles[k][h][:], a_tiles[k][h][:],
                        bt[:, h * HN : (h + 1) * HN],
                        op=mybir.AluOpType.add,
                    )

        def rhs_slice(k, nb):
            return a_tiles[k][nb // 2][:, (nb % 2) * 512 : (nb % 2 + 1) * 512]

        # ---- big matmul: out[(b,h), n] += xe^T @ A^T, bf16, f32 accum ----
        for pair in range(BH // 256):  # two (b,h) 128-chunks at a time
            pstiles = [
                [
                    ps_pool.tile(
                        [128, 512], F32, tag="ps", name=f"bps_{pair}_{h2}_{nb}"
                    )
                    for nb in range(NB)
                ]
                for h2 in range(2)
            ]
            if pair == 0:
                # arrival-paced: walk k outermost so each chunk is used
                # as soon as its A^T half lands
                for k in range(KC):
                    for half in range(2):
                        bh = 2 * pair + half
                        lhsT = xe_tiles[k][:, bh * 128 : (bh + 1) * 128]
                        for nb in range(NB):
                            nc.tensor.matmul(
                                pstiles[half][nb][:],
                                lhsT,
                                rhs_slice(k, nb),
                                start=(k == 0),
                                stop=(k == KC - 1),
                            )
            else:
                # all data has arrived by now: walk half outermost so the
                # first 4 groups close early and their copies/stores
                # overlap the remaining matmuls
                for half in range(2):
                    bh = 2 * pair + half
                    for nb in range(NB):
                        for k in range(KC):
                            nc.tensor.matmul(
                                pstiles[half][nb][:],
                                xe_tiles[k][:, bh * 128 : (bh + 1) * 128],
                                rhs_slice(k, nb),
                                start=(k == 0),
                                stop=(k == KC - 1),
                            )
                    for nb in range(NB):
                        ot = out_pool.tile(
                            [128, 512], F32, tag="o", name=f"ot1_{half}_{nb}"
                        )
                        if nb % 2 == 0:
                            nc.scalar.activation(
                                ot[:],
                                pstiles[half][nb][:],
                                mybir.ActivationFunctionType.Identity,
                            )
                        else:
                            nc.vector.tensor_copy(ot[:], pstiles[half][nb][:])
                        nc.scalar.dma_start(
                            outp[bh * 128 : (bh + 1) * 128,
                                 nb * 512 : (nb + 1) * 512],
                            ot[:],
                        )
                continue
            for half in range(2):
                bh = 2 * pair + half
                for nb in range(NB):
                    ot = out_pool.tile([128, 512], F32, tag="o")
                    if nb % 2 == 0:
                        nc.scalar.activation(
                            ot[:],
                            pstiles[half][nb][:],
                            mybir.ActivationFunctionType.Identity,
                        )
                    else:
                        nc.vector.tensor_copy(ot[:], pstiles[half][nb][:])
                    nc.sync.dma_start(
                        outp[bh * 128 : (bh + 1) * 128, nb * 512 : (nb + 1) * 512],
                        ot[:],
                    )

    nc.compile()
    return nc


def _get_program(with_bxes: bool, with_b: bool):
    key = (with_bxes, with_b)
    if key not in _PROGRAMS:
        _PROGRAMS[key] = _build_program(with_bxes, with_b)
    return _PROGRAMS[key]


def _prepare_in_maps(inputs, W_xes, b_xes, inci, w, b, with_bxes, with_b):
    bf16 = ml_dtypes.bfloat16
    inputs = np.asarray(inputs, dtype=np.float32)
    W_xes = np.asarray(W_xes, dtype=np.float32)
    b_xes = np.asarray(b_xes, dtype=np.float32)
    w = np.asarray(w, dtype=np.float32)
    b = np.asarray(b, dtype=np.float32)
    inci_u8 = np.asarray(inci).astype(np.uint8)

    wx_dup = np.zeros((128, 2 * DH), dtype=bf16)
    wx_dup[0:DIM, 0:DH] = W_xes.astype(bf16)
    wx_dup[DIM : 2 * DIM, DH : 2 * DH] = W_xes.astype(bf16)
    bxr = np.ascontiguousarray(
        np.broadcast_to(np.tile(b_xes, B)[None, :], (128, BH))
    ) if with_bxes else None

    in_maps = []
    for c in range(NCORES):
        sl = slice(c * EC, (c + 1) * EC)
        t = np.ascontiguousarray(
            inputs[:, sl, :].transpose(0, 2, 1)
        ).reshape(NJ, 128, EC).astype(bf16)
        wq_ = np.ascontiguousarray(w[:, sl].T).reshape(KC, 128, N).astype(bf16)
        iq_ = np.ascontiguousarray(inci_u8[:, sl].T).reshape(KC, 128, N)
        m = {"inp_t": t, "wq": wq_, "iq": iq_, "wx": wx_dup}
        if with_bxes:
            m["bxr"] = bxr
        if with_b:
            m["bq"] = np.ascontiguousarray(b[:, sl].T).reshape(
                KC, 128, N
            ).astype(bf16)
        in_maps.append(m)
    return in_maps


def _run(inputs, W_xes, b_xes, inci, w, b, **run_kwargs):
    with_bxes = bool(np.any(np.asarray(b_xes)))
    with_b = bool(np.any(np.asarray(b)))
    nc = _get_program(with_bxes, with_b)
    in_maps = _prepare_in_maps(inputs, W_xes, b_xes, inci, w, b, with_bxes, with_b)
    res = run_bass_kernel_spmd(
        nc, in_maps, core_ids=list(range(NCORES)), **run_kwargs
    )
    parts = np.stack([r["outp"] for r in res.results])  # [8, BH, N]
    out = parts.sum(axis=0)  # [BH, N]
    out = out.reshape(B, DH, N).transpose(0, 2, 1)  # [B, N, DH]
    return np.ascontiguousarray(out.astype(np.float32)), res


def kernel(inputs, W_xes, b_xes, inci, w, b):
    out, _ = _run(inputs, W_xes, b_xes, inci, w, b)
    return out


# revision 4
# speedup vs baseline: 1.6495x; 1.0514x over previous
"""Trainium2 Bass kernel for the NodeEdge GNN message-passing module.

Computes  out[b,n,h] = sum_e (w*inci + b)[n,e] * relu(inputs @ W_xes + b_xes)[b,e,h]
with B=16, N=2048, E=8192, DIM=64, DH=32.

Strategy: shard the edge (contraction) dimension E across the 8 NeuronCores
(EC=1024 edges per core). Each core:
  - computes xe = relu(inputs[:, e_shard, :] @ W_xes) for its edge shard
    in [e, (b,h)] layout via small PE matmuls,
  - forms A^T chunks (w * inci, transposed so e is the partition axis;
    the transpose itself is done on the host as layout prep),
  - runs the big matmul  out_partial[(b,h), n] = xe^T @ A^T  accumulating
    in f32 PSUM.
Partial outputs (one per core) are summed on the host.

All matmul operands travel as bf16 (PE runs bf16 at full rate vs half
rate for fp32r, and HBM traffic halves). inci is shipped as uint8 and
multiplied into the w chunks in place with a single mixed-dtype DVE
tensor_tensor (no separate cast pass, no GpSimd).
"""

from contextlib import ExitStack

import ml_dtypes
import numpy as np

import concourse.bass as bass
import concourse.mybir as mybir
import concourse.tile as tile
from concourse import bacc
from concourse.bass_utils import run_bass_kernel_spmd

B, N, E, DIM = 16, 2048, 8192, 64
DH = DIM // 2              # 32
NCORES = 8
EC = E // NCORES           # 1024 edges per core
KC = EC // 128             # 8 e-chunks of 128
BH = B * DH                # 512 (flattened (b, h) output dim)
NB = N // 512              # 4 column blocks of the big matmul
NJ = B // 2                # 8 input tiles, two batch rows packed per tile
HN = N // 2                # 1024, half-chunk width for w arrival pacing

F32 = mybir.dt.float32
BF16 = mybir.dt.bfloat16
U8 = mybir.dt.uint8

_PROGRAMS: dict = {}


def _build_program(with_bxes: bool, with_b: bool):
    nc = bacc.Bacc(
        "TRN2", target_bir_lowering=False, debug=False, enable_asserts=False
    )

    inp_t = nc.dram_tensor("inp_t", [NJ, 128, EC], BF16, kind="ExternalInput").ap()
    wq = nc.dram_tensor("wq", [KC, 128, N], BF16, kind="ExternalInput").ap()
    iq = nc.dram_tensor("iq", [KC, 128, N], U8, kind="ExternalInput").ap()
    wx = nc.dram_tensor("wx", [128, 2 * DH], BF16, kind="ExternalInput").ap()
    bxr = (
        nc.dram_tensor("bxr", [128, BH], F32, kind="ExternalInput").ap()
        if with_bxes
        else None
    )
    bq = (
        nc.dram_tensor("bq", [KC, 128, N], BF16, kind="ExternalInput").ap()
        if with_b
        else None
    )
    outp = nc.dram_tensor("outp", [BH, N], F32, kind="ExternalOutput").ap()

    with tile.TileContext(nc) as tc, ExitStack() as ctx:
        inp_pool = ctx.enter_context(tc.tile_pool(name="inp", bufs=NJ))
        wx_pool = ctx.enter_context(tc.tile_pool(name="wx", bufs=1))
        xe_pool = ctx.enter_context(tc.tile_pool(name="xe", bufs=KC))
        a_pool = ctx.enter_context(tc.tile_pool(name="a", bufs=2 * KC))
        i_pool = ctx.enter_context(tc.tile_pool(name="i", bufs=KC))
        out_pool = ctx.enter_context(tc.tile_pool(name="o", bufs=8))
        ps_pool = ctx.enter_context(tc.tile_pool(name="ps", bufs=8, space="PSUM"))

        # Block-diagonal xes weight: rows 0-63 map the even batch row to
        # output cols 0-31, rows 64-127 map the odd batch row to cols
        # 32-63, so one K=128 matmul computes xe for both packed batch
        # rows of an input tile at once.
        wx_tile = wx_pool.tile([128, 2 * DH], BF16)
        nc.sync.dma_start(wx_tile[:], wx[:])

        # ---- inputs first on the sync (HWDGE) queue: the xe matmul chain
        # is the head of the critical path, so these 256 KiB tiles must
        # land before anything else competes for HBM.
        inp_tiles = []
        for j in range(NJ):
            t = inp_pool.tile([128, EC], BF16)
            nc.sync.dma_start(t[:], inp_t[j])
            inp_tiles.append(t)

        # incidence loads early on the gpsimd (SWDGE) queue
        iu_tiles = []
        for k in range(KC):
            iu = i_pool.tile([128, N], U8, tag="iu", name=f"iu_{k}", bufs=KC)
            nc.gpsimd.dma_start(iu[:], iq[k])
            iu_tiles.append(iu)

        bx_tile = None
        if with_bxes:
            bx_tile = wx_pool.tile([128, BH], F32, tag="bx")
            nc.sync.dma_start(bx_tile[:], bxr[:])

        # ---- w chunks stream in as halves so the mask-multiply and the
        # first big matmuls start while later halves are still in flight.
        a_tiles = []
        for k in range(KC):
            ah = [
                a_pool.tile([128, HN], BF16, tag="a", bufs=2 * KC,
                            name=f"a_{k}_{h}")
                for h in range(2)
            ]
            for h in range(2):
                nc.sync.dma_start(ah[h][:], wq[k][:, h * HN : (h + 1) * HN])
            a_tiles.append(ah)

        # ---- xe = relu(inputs @ W_xes) in [e, (b,h)] layout ----
        xe_tiles = []
        for k in range(KC):
            ps = ps_pool.tile([128, BH], F32, tag="ps")
            for j in range(NJ):
                lhsT = inp_tiles[j][:, k * 128 : (k + 1) * 128]
                nc.tensor.matmul(
                    ps[:, j * 2 * DH : (j + 1) * 2 * DH],
                    lhsT,
                    wx_tile[:],
                    start=True,
                    stop=True,
                )
            xt = xe_pool.tile([128, BH], BF16)
            if with_bxes:
                nc.vector.tensor_tensor(
                    xt[:], ps[:], bx_tile[:], op=mybir.AluOpType.add
                )
                nc.scalar.activation(
                    xt[:], xt[:], mybir.ActivationFunctionType.Relu
                )
            else:
                nc.scalar.activation(
                    xt[:], ps[:], mybir.ActivationFunctionType.Relu
                )
            xe_tiles.append(xt)

        # ---- A^T chunks: a = w * inci (single mixed-dtype pass) ----
        for k in range(KC):
            for h in range(2):
                nc.vector.tensor_tensor(
                    a_tiles[k][h][:], a_tiles[k][h][:],
                    iu_tiles[k][:, h * HN : (h + 1) * HN],
                    op=mybir.AluOpType.mult,
                )
            if with_b:
                bt = i_pool.tile([128, N], BF16, tag="bt", bufs=2)
                nc.sync.dma_start(bt[:], bq[k])
                for h in range(2):
                    nc.vector.tensor_tensor(
                        a_tiles[k][h][:], a_tiles[k][h][:],
                        bt[:, h * HN : (h + 1) * HN],
                        op=mybir.AluOpType.add,
                    )

        def rhs_slice(k, nb):
            return a_tiles[k][nb // 2][:, (nb % 2) * 512 : (nb % 2 + 1) * 512]

        # ---- big matmul: out[(b,h), n] += xe^T @ A^T, bf16, f32 accum ----
        # Within each (k, bh) the 4 nb matmuls share the same stationary
        # operand, so only the first self-loads weights (ldweights=False
        # on the rest shaves the ~110ns LDWEIGHTS off 3 of every 4 MMs).
        def store_group(pair, half, pstiles, dma_engine):
            bh = 2 * pair + half
            for nb in range(NB):
                ot = out_pool.tile(
                    [128, 512], F32, tag="o", name=f"ot_{pair}_{half}_{nb}"
                )
                if nb % 2 == 0:
                    nc.scalar.activation(
                        ot[:],
                        pstiles[half][nb][:],
                        mybir.ActivationFunctionType.Identity,
                    )
                else:
                    nc.vector.tensor_copy(ot[:], pstiles[half][nb][:])
                dma_engine.dma_start(
                    outp[bh * 128 : (bh + 1) * 128, nb * 512 : (nb + 1) * 512],
                    ot[:],
                )

        for pair in range(BH // 256):  # two (b,h) 128-chunks at a time
            pstiles = [
                [
                    ps_pool.tile(
                        [128, 512], F32, tag="ps", name=f"bps_{pair}_{h2}_{nb}"
                    )
                    for nb in range(NB)
                ]
                for h2 in range(2)
            ]
            if pair == 0:
                # arrival-paced: walk k outermost so each chunk is used
                # as soon as its A^T half lands
                for k in range(KC):
                    for half in range(2):
                        bh = 2 * pair + half
                        lhsT = xe_tiles[k][:, bh * 128 : (bh + 1) * 128]
                        for nb in range(NB):
                            mm = nc.tensor.matmul(
                                pstiles[half][nb][:],
                                lhsT,
                                rhs_slice(k, nb),
                                start=(k == 0),
                                stop=(k == KC - 1),
                            )
                            if nb > 0:
                                mm.ins.ldweights = False
            else:
                # all data has arrived by now: walk half outermost so the
                # first 4 groups close early and their copies/stores
                # overlap the remaining matmuls
                for half in range(2):
                    bh = 2 * pair + half
                    for k in range(KC):
                        lhsT = xe_tiles[k][:, bh * 128 : (bh + 1) * 128]
                        for nb in range(NB):
                            mm = nc.tensor.matmul(
                                pstiles[half][nb][:],
                                lhsT,
                                rhs_slice(k, nb),
                                start=(k == 0),
                                stop=(k == KC - 1),
                            )
                            if nb > 0:
                                mm.ins.ldweights = False
                    store_group(pair, half, pstiles, nc.scalar)
                continue
            # pair-0 groups close first: store them immediately so the
            # copies and output DMAs overlap pair-1's matmuls
            for half in range(2):
                store_group(0, half, pstiles, nc.sync)

    nc.compile()
    return nc


def _get_program(with_bxes: bool, with_b: bool):
    key = (with_bxes, with_b)
    if key not in _PROGRAMS:
        _PROGRAMS[key] = _build_program(with_bxes, with_b)
    return _PROGRAMS[key]


def _prepare_in_maps(inputs, W_xes, b_xes, inci, w, b, with_bxes, with_b):
    bf16 = ml_dtypes.bfloat16
    inputs = np.asarray(inputs, dtype=np.float32)
    W_xes = np.asarray(W_xes, dtype=np.float32)
    b_xes = np.asarray(b_xes, dtype=np.float32)
    w = np.asarray(w, dtype=np.float32)
    b = np.asarray(b, dtype=np.float32)
    inci_u8 = np.asarray(inci).astype(np.uint8)

    wx_dup = np.zeros((128, 2 * DH), dtype=bf16)
    wx_dup[0:DIM, 0:DH] = W_xes.astype(bf16)
    wx_dup[DIM : 2 * DIM, DH : 2 * DH] = W_xes.astype(bf16)
    bxr = np.ascontiguousarray(
        np.broadcast_to(np.tile(b_xes, B)[None, :], (128, BH))
    ) if with_bxes else None

    in_maps = []
    for c in range(NCORES):
        sl = slice(c * EC, (c + 1) * EC)
        t = np.ascontiguousarray(
            inputs[:, sl, :].transpose(0, 2, 1)
        ).reshape(NJ, 128, EC).astype(bf16)
        wq_ = np.ascontiguousarray(w[:, sl].T).reshape(KC, 128, N).astype(bf16)
        iq_ = np.ascontiguousarray(inci_u8[:, sl].T).reshape(KC, 128, N)
        m = {"inp_t": t, "wq": wq_, "iq": iq_, "wx": wx_dup}
        if with_bxes:
            m["bxr"] = bxr
        if with_b:
            m["bq"] = np.ascontiguousarray(b[:, sl].T).reshape(
                KC, 128, N
            ).astype(bf16)
        in_maps.append(m)
    return in_maps


def _run(inputs, W_xes, b_xes, inci, w, b, **run_kwargs):
    with_bxes = bool(np.any(np.asarray(b_xes)))
    with_b = bool(np.any(np.asarray(b)))
    nc = _get_program(with_bxes, with_b)
    in_maps = _prepare_in_maps(inputs, W_xes, b_xes, inci, w, b, with_bxes, with_b)
    res = run_bass_kernel_spmd(
        nc, in_maps, core_ids=list(range(NCORES)), **run_kwargs
    )
    parts = np.stack([r["outp"] for r in res.results])  # [8, BH, N]
    out = parts.sum(axis=0)  # [BH, N]
    out = out.reshape(B, DH, N).transpose(0, 2, 1)  # [B, N, DH]
    return np.ascontiguousarray(out.astype(np.float32)), res


def kernel(inputs, W_xes, b_xes, inci, w, b):
    out, _ = _run(inputs, W_xes, b_xes, inci, w, b)
    return out


# revision 5
# speedup vs baseline: 1.6640x; 1.0088x over previous
"""Trainium2 Bass kernel for the NodeEdge GNN message-passing module.

Computes  out[b,n,h] = sum_e (w*inci + b)[n,e] * relu(inputs @ W_xes + b_xes)[b,e,h]
with B=16, N=2048, E=8192, DIM=64, DH=32.

Strategy: shard the edge (contraction) dimension E across the 8 NeuronCores
(EC=1024 edges per core). Each core:
  - computes xe = relu(inputs[:, e_shard, :] @ W_xes) for its edge shard
    in [e, (b,h)] layout via small PE matmuls,
  - forms A^T chunks (w * inci, transposed so e is the partition axis;
    the transpose itself is done on the host as layout prep),
  - runs the big matmul  out_partial[(b,h), n] = xe^T @ A^T  accumulating
    in f32 PSUM.
Partial outputs (one per core, bf16) are summed on the host in f32.

All matmul operands travel as bf16 (full PE rate, half the HBM bytes of
f32). inci ships as uint8 and is multiplied into the w chunks in place
with one mixed-dtype DVE tensor_tensor. Inputs are repacked k-major on
the host so each xe chunk depends on a single 256 KiB DMA, and all DRAM
rows are >= 4 KiB so the HWDGE descriptor rate doesn't cap queue
bandwidth. The in-order sync queue orders inputs ahead of w chunks; the
incidence stream rides the gpsimd queue concurrently.
"""

from contextlib import ExitStack

import ml_dtypes
import numpy as np

import concourse.bass as bass
import concourse.mybir as mybir
import concourse.tile as tile
from concourse import bacc
from concourse.bass_utils import run_bass_kernel_spmd

B, N, E, DIM = 16, 2048, 8192, 64
DH = DIM // 2              # 32
NCORES = 8
EC = E // NCORES           # 1024 edges per core
KC = EC // 128             # 8 e-chunks of 128
KP = KC // 2               # 4 pair tiles (two e-chunks per DMA)
BH = B * DH                # 512 (flattened (b, h) output dim)
NB = N // 512              # 4 column blocks of the big matmul
NJ = B // 2                # 8 lhsT blocks per e-chunk (two batch rows each)

F32 = mybir.dt.float32
BF16 = mybir.dt.bfloat16
U8 = mybir.dt.uint8

_PROGRAMS: dict = {}


def _build_program(with_bxes: bool, with_b: bool):
    nc = bacc.Bacc(
        "TRN2", target_bir_lowering=False, debug=False, enable_asserts=False
    )

    # k-major inputs: pair tile kk holds chunks 2kk,2kk+1; within a chunk,
    # NJ lhsT blocks of [128 (2b,d), 128 e] side by side.
    inp_t = nc.dram_tensor("inp_t", [KP, 128, 2 * EC], BF16, kind="ExternalInput").ap()
    wq = nc.dram_tensor("wq", [KC, 128, N], BF16, kind="ExternalInput").ap()
    iq = nc.dram_tensor("iq", [KP, 128, 2 * N], U8, kind="ExternalInput").ap()
    wx = nc.dram_tensor("wx", [128, 2 * DH], BF16, kind="ExternalInput").ap()
    bxr = (
        nc.dram_tensor("bxr", [128, BH], F32, kind="ExternalInput").ap()
        if with_bxes
        else None
    )
    bq = (
        nc.dram_tensor("bq", [KC, 128, N], BF16, kind="ExternalInput").ap()
        if with_b
        else None
    )
    outp = nc.dram_tensor("outp", [BH, N], BF16, kind="ExternalOutput").ap()

    with tile.TileContext(nc) as tc, ExitStack() as ctx:
        inp_pool = ctx.enter_context(tc.tile_pool(name="inp", bufs=KP))
        wx_pool = ctx.enter_context(tc.tile_pool(name="wx", bufs=1))
        xe_pool = ctx.enter_context(tc.tile_pool(name="xe", bufs=KC))
        a_pool = ctx.enter_context(tc.tile_pool(name="a", bufs=KC))
        i_pool = ctx.enter_context(tc.tile_pool(name="i", bufs=KP))
        out_pool = ctx.enter_context(tc.tile_pool(name="o", bufs=4))
        ps_pool = ctx.enter_context(tc.tile_pool(name="ps", bufs=8, space="PSUM"))

        # Block-diagonal xes weight: rows 0-63 map the even batch row to
        # output cols 0-31, rows 64-127 map the odd batch row to cols
        # 32-63, so one K=128 matmul computes xe for both packed batch
        # rows of an input tile at once.
        wx_tile = wx_pool.tile([128, 2 * DH], BF16)
        nc.sync.dma_start(wx_tile[:], wx[:])

        # inputs head the in-order sync queue: the xe chain is the start
        # of the critical path
        inp_tiles = []
        for kk in range(KP):
            t = inp_pool.tile([128, 2 * EC], BF16)
            nc.sync.dma_start(t[:], inp_t[kk])
            inp_tiles.append(t)

        # incidence streams concurrently on the gpsimd queue
        iu_tiles = []
        for kk in range(KP):
            iu = i_pool.tile([128, 2 * N], U8, tag="iu", name=f"iu_{kk}", bufs=KP)
            nc.gpsimd.dma_start(iu[:], iq[kk])
            iu_tiles.append(iu)

        bx_tile = None
        if with_bxes:
            bx_tile = wx_pool.tile([128, BH], F32, tag="bx")
            nc.sync.dma_start(bx_tile[:], bxr[:])

        # w chunks queue behind the inputs on sync (in-order = priority)
        a_tiles = []
        for k in range(KC):
            at = a_pool.tile([128, N], BF16, tag="a", bufs=KC, name=f"a_{k}")
            nc.sync.dma_start(at[:], wq[k])
            a_tiles.append(at)

        # ---- xe = relu(inputs @ W_xes) in [e, (b,h)] layout ----
        xe_tiles = []
        for k in range(KC):
            base = (k % 2) * EC
            ps = ps_pool.tile([128, BH], F32, tag="ps")
            for j in range(NJ):
                lhsT = inp_tiles[k // 2][:, base + j * 128 : base + (j + 1) * 128]
                nc.tensor.matmul(
                    ps[:, j * 2 * DH : (j + 1) * 2 * DH],
                    lhsT,
                    wx_tile[:],
                    start=True,
                    stop=True,
                )
            xt = xe_pool.tile([128, BH], BF16)
            if with_bxes:
                nc.vector.tensor_tensor(
                    xt[:], ps[:], bx_tile[:], op=mybir.AluOpType.add
                )
                nc.scalar.activation(
                    xt[:], xt[:], mybir.ActivationFunctionType.Relu
                )
            else:
                nc.scalar.activation(
                    xt[:], ps[:], mybir.ActivationFunctionType.Relu
                )
            xe_tiles.append(xt)

        # ---- A^T chunks: a = w * inci (single mixed-dtype pass) ----
        for k in range(KC):
            nc.vector.tensor_tensor(
                a_tiles[k][:], a_tiles[k][:],
                iu_tiles[k // 2][:, (k % 2) * N : (k % 2 + 1) * N],
                op=mybir.AluOpType.mult,
            )
            if with_b:
                bt = i_pool.tile([128, N], BF16, tag="bt", bufs=2)
                nc.sync.dma_start(bt[:], bq[k])
                nc.vector.tensor_tensor(
                    a_tiles[k][:], a_tiles[k][:], bt[:],
                    op=mybir.AluOpType.add,
                )

        # ---- big matmul: out[(b,h), n] += xe^T @ A^T, bf16, f32 accum ----
        # stores pack two nb blocks into one [128,1024] tile so the DRAM
        # write rows stay at 2 KiB
        def store_pair(pair, half, pstiles, nbp, dma_engine):
            bh = 2 * pair + half
            ot = out_pool.tile(
                [128, 1024], BF16, tag="o", name=f"ot_{pair}_{half}_{nbp}"
            )
            for i in range(2):
                nb = 2 * nbp + i
                sl = ot[:, i * 512 : (i + 1) * 512]
                if i == 0:
                    nc.scalar.activation(
                        sl, pstiles[half][nb][:],
                        mybir.ActivationFunctionType.Identity,
                    )
                else:
                    nc.vector.tensor_copy(sl, pstiles[half][nb][:])
            dma_engine.dma_start(
                outp[bh * 128 : (bh + 1) * 128, nbp * 1024 : (nbp + 1) * 1024],
                ot[:],
            )

        for pair in range(BH // 256):  # two (b,h) 128-chunks at a time
            pstiles = [
                [
                    ps_pool.tile(
                        [128, 512], F32, tag="ps", name=f"bps_{pair}_{h2}_{nb}"
                    )
                    for nb in range(NB)
                ]
                for h2 in range(2)
            ]
            if pair == 0:
                # arrival-paced: walk k outermost so each chunk is used
                # as soon as its A^T tile is ready
                for k in range(KC):
                    for half in range(2):
                        bh = 2 * pair + half
                        lhsT = xe_tiles[k][:, bh * 128 : (bh + 1) * 128]
                        for nb in range(NB):
                            nc.tensor.matmul(
                                pstiles[half][nb][:],
                                lhsT,
                                a_tiles[k][:, nb * 512 : (nb + 1) * 512],
                                start=(k == 0),
                                stop=(k == KC - 1),
                            )
                for half in range(2):
                    for nbp in range(NB // 2):
                        store_pair(0, half, pstiles, nbp, nc.gpsimd)
            else:
                # all data resident: close each nb group after its 8 MMs
                # so stores pipeline with the remaining matmuls
                for half in range(2):
                    bh = 2 * pair + half
                    for nb in range(NB):
                        for k in range(KC):
                            nc.tensor.matmul(
                                pstiles[half][nb][:],
                                xe_tiles[k][:, bh * 128 : (bh + 1) * 128],
                                a_tiles[k][:, nb * 512 : (nb + 1) * 512],
                                start=(k == 0),
                                stop=(k == KC - 1),
                            )
                        if nb % 2 == 1:
                            store_pair(
                                pair, half, pstiles, nb // 2,
                                nc.scalar if half == 0 else nc.sync,
                            )

    nc.compile()
    return nc


def _get_program(with_bxes: bool, with_b: bool):
    key = (with_bxes, with_b)
    if key not in _PROGRAMS:
        _PROGRAMS[key] = _build_program(with_bxes, with_b)
    return _PROGRAMS[key]


def _prepare_in_maps(inputs, W_xes, b_xes, inci, w, b, with_bxes, with_b):
    bf16 = ml_dtypes.bfloat16
    inputs = np.asarray(inputs, dtype=np.float32)
    W_xes = np.asarray(W_xes, dtype=np.float32)
    b_xes = np.asarray(b_xes, dtype=np.float32)
    w = np.asarray(w, dtype=np.float32)
    b = np.asarray(b, dtype=np.float32)
    inci_u8 = np.asarray(inci).astype(np.uint8)

    wx_dup = np.zeros((128, 2 * DH), dtype=bf16)
    wx_dup[0:DIM, 0:DH] = W_xes.astype(bf16)
    wx_dup[DIM : 2 * DIM, DH : 2 * DH] = W_xes.astype(bf16)
    bxr = np.ascontiguousarray(
        np.broadcast_to(np.tile(b_xes, B)[None, :], (128, BH))
    ) if with_bxes else None

    in_maps = []
    for c in range(NCORES):
        sl = slice(c * EC, (c + 1) * EC)
        # k-major inputs: [j, b2, k, e, d] -> [k, (b2,d), j, e]
        t = inputs[:, sl, :].reshape(NJ, 2, KC, 128, DIM)
        t = np.ascontiguousarray(t.transpose(2, 1, 4, 0, 3)).reshape(
            KC, 128, EC
        )
        # pair tiles: [kk, (b2,d), 2*EC]
        t = np.ascontiguousarray(
            t.reshape(KP, 2, 128, EC).transpose(0, 2, 1, 3)
        ).reshape(KP, 128, 2 * EC).astype(bf16)
        wq_ = np.ascontiguousarray(w[:, sl].T).reshape(KC, 128, N).astype(bf16)
        iq_ = np.ascontiguousarray(
            inci_u8[:, sl].T.reshape(KP, 2, 128, N).transpose(0, 2, 1, 3)
        ).reshape(KP, 128, 2 * N)
        m = {"inp_t": t, "wq": wq_, "iq": iq_, "wx": wx_dup}
        if with_bxes:
            m["bxr"] = bxr
        if with_b:
            m["bq"] = np.ascontiguousarray(b[:, sl].T).reshape(
                KC, 128, N
            ).astype(bf16)
        in_maps.append(m)
    return in_maps


def _run(inputs, W_xes, b_xes, inci, w, b, **run_kwargs):
    with_bxes = bool(np.any(np.asarray(b_xes)))
    with_b = bool(np.any(np.asarray(b)))
    nc = _get_program(with_bxes, with_b)
    in_maps = _prepare_in_maps(inputs, W_xes, b_xes, inci, w, b, with_bxes, with_b)
    res = run_bass_kernel_spmd(
        nc, in_maps, core_ids=list(range(NCORES)), **run_kwargs
    )
    parts = np.stack(
        [np.asarray(r["outp"], dtype=np.float32) for r in res.results]
    )  # [8, BH, N] f32
    out = parts.sum(axis=0)  # [BH, N]
    out = out.reshape(B, DH, N).transpose(0, 2, 1)  # [B, N, DH]
    return np.ascontiguousarray(out.astype(np.float32)), res


def kernel(inputs, W_xes, b_xes, inci, w, b):
    out, _ = _run(inputs, W_xes, b_xes, inci, w, b)
    return out
